# revision 1
# baseline (speedup 1.0000x reference)
"""GPT-2 style causal attention block (B=4, S=2048, E=1024, H=16, D=64) on
8 TRN2 NeuronCores.

Sharding: batch(4) x head-half(2) -> 8 cores, zero on-device communication.
Core c handles batch b=c//2 and heads h0=(c%2)*8 .. h0+7. Each core computes
its qkv column block, attention for its 8 heads, and a partial c_proj
(its 512 rows of w_proj). The two partial outputs per batch are summed on the
host during unshard (b_proj is given only to the even core of each pair).

On-device layout (per core, everything transposed so the softmax reduction
lands on the PE via a ones-column appended to each head's V block):
  X^T [E, S]     host-pre-transposed, DMA'd directly as bf16
  Q^T,K^T [1024, S]  W-stationary matmuls; V in [S, 520] ([64 d | 1] per head,
  the ones-columns written once by a strided memset)
  scores^T[k, q] per head in 1024-wide q-chunks; exp on ACT (1/sqrt(D) folded
  into the act scale); causality by computing only k<=q 128-tiles plus a
  gpsimd affine_select on each diagonal 128-block
  attn@V accumulates out^T[64+1, q] in PSUM; row 64 = softmax denominator
  c_proj from A^T [512, S] with this core's W_proj rows -> partial y [S, E];
  tail-region tiles stream per-head-pair partials to out2 (host-summed)

Scheduling: attention bodies priority-boosted over filler (qkv pairs 2-3,
V tiles 8-15, c_proj tiles) which is interleaved into the exp-bound
stretches; during the DMA-paced ramp the qkv groups borrow the idle
attention PSUM banks. Host converts inputs to bf16; PSUM accumulates f32;
bf16 outputs are upcast and pair-summed on the host.
"""

import re

import ml_dtypes
import numpy as np

import concourse.mybir as mybir
import concourse.tile as tile
from concourse import bacc
from concourse.bass_utils import run_bass_kernel_spmd
from concourse.vector_clock import ScopedClock

F32 = mybir.dt.float32
BF16 = mybir.dt.bfloat16
BF16_NP = ml_dtypes.bfloat16
AF = mybir.ActivationFunctionType

S = 2048          # sequence length (per batch)
E = 1024          # embedding dim
HL = 8            # heads per core
D = 64            # head dim
TT = S // 128     # 16 token tiles
ET = E // 128     # 8 embedding tiles
NCH = S // 1024   # 2 q-chunks of 1024
VW = HL * (D + 1)  # 520: V block width with per-head ones-column
PRIO_OFFSET = 800  # attention body scheduled ahead of filler work


def _install_drain_fix():
    """walrus in this container rejects the Tile kernel-tail Drain when it
    carries all semaphore waits on one instruction ("Too many sync wait
    commands"). Emit one wait_ge per semaphore, then a bare drain."""
    if getattr(tile.TileContext, "_drain_fix_installed", False):
        return

    def _split_drain_and_barrier(self, tick_clock, wait_clock):
        nc = self.nc
        probe = mybir.InstDrain(
            name="probe-drain", engine=mybir.EngineType.SP, ins=[], outs=[]
        )
        wait_clock.add_sem_waits(probe, ScopedClock({None: tick_clock.global_clock}))
        waits = re.findall(r"wait:S\[([A-Za-z0-9_]+)\]>=(\d+)", probe.concise())
        handles = {h.name: h for h in self.sems.allocated().values()}
        for name, val in waits:
            nc.sync.wait_ge(handles[name], int(val))
        nc.sync.drain()
        nc.all_engine_barrier()
        popped = nc._tile_sem_poison_stack.pop()
        assert popped is self._sem_poison
        nc.clear_and_free_semaphores(list(self.sems.allocated().values()))
        nc.all_engine_barrier()

    tile.TileContext._drain_and_barrier = _split_drain_and_barrier
    tile.TileContext._drain_fix_installed = True


def _emit(nc, tc, ctx):
    xt_d = nc.declare_dram_parameter("xt", [E, S], BF16, isOutput=False)
    wqk_d = nc.declare_dram_parameter("wqk", [E, 1024], BF16, isOutput=False)
    wva_d = nc.declare_dram_parameter("wva", [E, 512], BF16, isOutput=False)
    wp_d = nc.declare_dram_parameter("wp", [512, E], BF16, isOutput=False)
    bqk_d = nc.declare_dram_parameter("bqk", [8, 128, 1], F32, isOutput=False)
    bva_d = nc.declare_dram_parameter("bva", [1, 512], F32, isOutput=False)
    bp_d = nc.declare_dram_parameter("bp", [1, E], F32, isOutput=False)
    out_d = nc.declare_dram_parameter("out", [S, E], BF16, isOutput=True)
    # tail-region (rows 1024:2048) c_proj partials, one per head pair;
    # summed on the host together with the core-pair reduction
    out2_d = nc.declare_dram_parameter("out2", [4, 1024, E], BF16, isOutput=True)

    consts = ctx.enter_context(tc.tile_pool(name="consts", bufs=1))
    statics = ctx.enter_context(tc.tile_pool(name="statics", bufs=1))
    ptp = ctx.enter_context(tc.tile_pool(name="ptp", bufs=6))
    rp = ctx.enter_context(tc.tile_pool(name="rp", bufs=2))
    yp = ctx.enter_context(tc.tile_pool(name="yp", bufs=3))
    # PSUM budget (8 banks): sc 2x[128,1024]=4, o 1x[65,1024]=2, qk 2x[128,512]=2
    psS = ctx.enter_context(tc.tile_pool(name="psS", bufs=2, space="PSUM"))
    psO = ctx.enter_context(tc.tile_pool(name="psO", bufs=1, space="PSUM"))
    psQ = ctx.enter_context(tc.tile_pool(name="psQ", bufs=2, space="PSUM"))

    # ---- front section: DMA order matters (the DMA engines are a single
    # serialized 360GB/s resource). X^T arrives host-pre-transposed; wqk
    # interleaves so qkv unlocks early; wp (needed last) at the end ----
    xt_sb = statics.tile([128, ET, S], BF16)
    wqk_sb = statics.tile([128, ET, 1024], BF16)
    wva_sb = statics.tile([128, ET, 512], BF16)
    wp_sb = statics.tile([128, 4, E], BF16)

    for et in range(ET):
        if et == 0:
            nc.sync.dma_start(
                out=xt_sb[:, 0, 0:1024], in_=xt_d[0:128, 0:1024]
            )
            nc.sync.dma_start(
                out=xt_sb[:, 0, 1024:S], in_=xt_d[0:128, 1024:S]
            )
        else:
            nc.sync.dma_start(
                out=xt_sb[:, et, :], in_=xt_d[et * 128 : (et + 1) * 128, :]
            )
        nc.gpsimd.dma_start(
            out=wqk_sb[:, et, :], in_=wqk_d[et * 128 : (et + 1) * 128, :]
        )
    for et in range(ET):
        nc.gpsimd.dma_start(
            out=wva_sb[:, et, :], in_=wva_d[et * 128 : (et + 1) * 128, :]
        )
    for ct in range(4):
        nc.gpsimd.dma_start(out=wp_sb[:, ct, :], in_=wp_d[ct * 128 : (ct + 1) * 128, :])

    bqk_sb = consts.tile([128, 8], F32)
    for m in range(8):
        nc.sync.dma_start(out=bqk_sb[:, m : m + 1], in_=bqk_d[m])
    bva_st = consts.tile([1, 512], F32)
    nc.sync.dma_start(out=bva_st, in_=bva_d[:])
    bva_bc = consts.tile([128, 512], F32)
    nc.gpsimd.partition_broadcast(out_ap=bva_bc[:], in_ap=bva_st[:])
    bp_st = consts.tile([1, E], F32)
    nc.sync.dma_start(out=bp_st, in_=bp_d[:])
    bp_bc = consts.tile([128, E], F32)
    nc.gpsimd.partition_broadcast(out_ap=bp_bc[:], in_ap=bp_st[:])

    # ---- qkv Q^T,K^T (W stationary) paired so head h's Q and K m-tiles
    # arrive together, interleaved with V tiles -> attention starts early ----
    qkt_sb = statics.tile([128, 8, S], BF16)
    va_sb = statics.tile([128, TT, VW], BF16)
    # per-head ones columns (position 64 of each 65-wide head block), written
    # once; emit_v scatters only the 64 d-cols per head via a strided AP
    nc.gpsimd.memset(
        va_sb[:, :, :].rearrange("p i (h c) -> p i h c", c=D + 1)[:, :, :, D : D + 1],
        1.0,
    )

    def emit_qk(m, ramp=False):
        # during the DMA-paced ramp the attention PSUM banks are still free:
        # spread the first pair's groups across them so more et-accumulations
        # are in flight per arriving weight tile
        pools = [psQ, psQ, psS, psO] if ramp else [psQ] * 4
        tags = ["qk", "qk", "sc", "o"] if ramp else ["qk"] * 4
        for tch in range(4):
            pqk = pools[tch].tile([128, 512], F32, tag=tags[tch])
            for et in range(ET):
                nc.tensor.matmul(
                    pqk,
                    wqk_sb[:, et, m * 128 : (m + 1) * 128],
                    xt_sb[:, et, tch * 512 : (tch + 1) * 512],
                    start=(et == 0),
                    stop=(et == ET - 1),
                )
            nc.vector.tensor_scalar_add(
                qkt_sb[:, m, tch * 512 : (tch + 1) * 512], pqk, bqk_sb[:, m : m + 1]
            )

    def emit_v(i, ramp=False):
        pv1 = (psS if ramp else psQ).tile([128, 512], F32, tag="sc" if ramp else "qk")
        for et in range(ET):
            nc.tensor.matmul(
                pv1,
                xt_sb[:, et, i * 128 : (i + 1) * 128],
                wva_sb[:, et, :],
                start=(et == 0),
                stop=(et == ET - 1),
            )
        nc.vector.tensor_add(
            va_sb[:, i, :].rearrange("p (h c) -> p h c", c=D + 1)[:, :, 0:D],
            pv1[:, :].rearrange("p (h c) -> p h c", c=D),
            bva_bc[:, :].rearrange("p (h c) -> p h c", c=D),
        )

    # Q/K pairs 0-1 and V tiles 0-7 up front (enough for chunk-0 heads 0-3);
    # pairs 2-3 are deferred into the chunk-0 head loop as PE filler, arriving
    # three heads before their consumers
    for m in range(2):
        emit_qk(m, ramp=True)      # Q m-tile: heads 2m, 2m+1
        emit_qk(4 + m, ramp=True)  # K m-tile: heads 2m, 2m+1
        emit_v(4 * m, ramp=True)
        emit_v(4 * m + 1, ramp=True)
        emit_v(4 * m + 2)
        emit_v(4 * m + 3)

    # ---- attention (q-chunks of 1024), interleaved with c_proj halves ----
    at_sb = statics.tile([128, 4, S], BF16)  # A^T: rows c=h*64+d, cols t

    def segs(off):
        if off < 512:
            return [(off, 512), (512, 1024)]
        return [(off, 1024)]

    def emit_cproj(i):
        ysb = yp.tile([128, E], BF16, tag="y")
        for ech in range(2):
            py = psQ.tile([128, 512], F32, tag="qk")
            for ct in range(4):
                nc.tensor.matmul(
                    py,
                    at_sb[:, ct, i * 128 : (i + 1) * 128],
                    wp_sb[:, ct, ech * 512 : (ech + 1) * 512],
                    start=(ct == 0),
                    stop=(ct == 3),
                )
            nc.vector.tensor_add(
                ysb[:, ech * 512 : (ech + 1) * 512],
                py,
                bp_bc[:, ech * 512 : (ech + 1) * 512],
            )
            nc.sync.dma_start(
                out=out_d[i * 128 : (i + 1) * 128, ech * 512 : (ech + 1) * 512],
                in_=ysb[:, ech * 512 : (ech + 1) * 512],
            )

    for j in range(NCH):
        q0 = j * 1024
        nkt = 8 * (j + 1)
        # attn@V piece list per kt, with PSUM group flags: the sim (and HW
        # pending-zero) track groups per 2KB bank keyed by each matmul's
        # START byte — the first piece starting in a bank carries start=True
        # (marks the whole bank pending-zero), the last carries stop=True.
        av_pieces = []  # (kt, a, b)
        for kt in range(nkt):
            p = kt - 8 * j
            off = max(0, p * 128)
            for a, b in segs(off):
                av_pieces.append((kt, a, b))
        first_in_bank, last_in_bank = {}, {}
        for idx, (kt, a, b) in enumerate(av_pieces):
            bank = a // 512
            first_in_bank.setdefault(bank, idx)
            last_in_bank[bank] = idx
        starts = set(first_in_bank.values())
        stops = set(last_in_bank.values())

        for h in range(HL):
            po = (h % 2) * 64
            qm, km = h // 2, 4 + h // 2
            hp = ctx_hp = tc.high_priority(offset=PRIO_OFFSET)
            ctx_hp.__enter__()
            pso = psO.tile([65, 1024], F32, tag="o")
            # drain each pso bank to SBUF right after its last av write so
            # the single psO slot frees as early as possible
            osb = rp.tile([65, 1024], F32, tag="os")
            idx = 0
            for kt in range(nkt):
                p = kt - 8 * j
                off = max(0, p * 128)
                ps2 = psS.tile([128, 1024], F32, tag="sc")
                for a, b in segs(off):
                    nc.tensor.matmul(
                        ps2[:, a:b],
                        qkt_sb[po : po + 64, km, kt * 128 : (kt + 1) * 128],
                        qkt_sb[po : po + 64, qm, q0 + a : q0 + b],
                        start=True,
                        stop=True,
                    )
                pt = ptp.tile([128, 1024], BF16, tag="pt")
                nc.scalar.activation(
                    out=pt[:, off:1024], in_=ps2[:, off:1024], func=AF.Exp, scale=0.125
                )
                if p >= 0:
                    # causal triangle on the diagonal 128-block: keep where
                    # q >= k, zero elsewhere (Pool engine; DVE is busier)
                    nc.gpsimd.affine_select(
                        out=pt[:, off : off + 128],
                        in_=pt[:, off : off + 128],
                        compare_op=mybir.AluOpType.is_ge,
                        fill=0.0,
                        base=0,
                        pattern=[[1, 128]],
                        channel_multiplier=-1,
                    )
                while idx < len(av_pieces) and av_pieces[idx][0] == kt:
                    _, a, b = av_pieces[idx]
                    nc.tensor.matmul(
                        pso[:, a:b],
                        va_sb[:, kt, h * 65 : (h + 1) * 65],
                        pt[:, a:b],
                        start=(idx in starts),
                        stop=(idx in stops),
                    )
                    if idx == last_in_bank[0]:
                        nc.vector.tensor_copy(osb[:, 0:512], pso[:, 0:512])
                    elif idx == last_in_bank[1]:
                        nc.vector.tensor_copy(osb[:, 512:1024], pso[:, 512:1024])
                    idx += 1
            rinv = rp.tile([1, 1024], F32, tag="ri")
            rbc = rp.tile([64, 1024], F32, tag="rb")
            for z in range(2):
                zs = slice(z * 512, (z + 1) * 512)
                nc.vector.reciprocal(out=rinv[:, zs], in_=osb[64:65, zs])
                nc.gpsimd.partition_broadcast(
                    out_ap=rbc[:, zs], in_ap=rinv[:, zs]
                )
                nc.vector.tensor_mul(
                    at_sb[po : po + 64, h // 2, q0 + z * 512 : q0 + (z + 1) * 512],
                    osb[0:64, zs],
                    rbc[:, zs],
                )
            ctx_hp.__exit__(None, None, None)
            if j == 0:
                # V tiles 8-15 (needed only by chunk 1) as PE filler while
                # chunk-0 attention is ACT(exp)-rate-bound
                emit_v(8 + h)
                if h == 1:
                    emit_qk(2)
                    emit_qk(6)
                elif h == 3:
                    emit_qk(3)
                    emit_qk(7)
            else:
                # chunk-0 c_proj tiles as PE filler for chunk-1 attention
                emit_cproj(h)
                if h % 2 == 1:
                    # tail-region (tiles 8-15) c_proj partial for the head
                    # pair that just finished: ct-slice head-pair h//2. Runs
                    # as PE filler instead of serializing after the last
                    # head; the host sums the 4 partials (it already sums
                    # core pairs).
                    ct = h // 2
                    for i in range(8, 16):
                        y2 = yp.tile([128, E], BF16, tag="y")
                        for ech in range(2):
                            py = psQ.tile([128, 512], F32, tag="qk")
                            nc.tensor.matmul(
                                py,
                                at_sb[:, ct, i * 128 : (i + 1) * 128],
                                wp_sb[:, ct, ech * 512 : (ech + 1) * 512],
                                start=True,
                                stop=True,
                            )
                            if ct == 0:
                                nc.vector.tensor_add(
                                    y2[:, ech * 512 : (ech + 1) * 512],
                                    py,
                                    bp_bc[:, ech * 512 : (ech + 1) * 512],
                                )
                            elif ct == 3:
                                # tail pair: alternate ACT/DVE so the copy
                                # rate matches the matmul rate
                                if (i + ech) % 2 == 0:
                                    nc.scalar.copy(
                                        out=y2[:, ech * 512 : (ech + 1) * 512],
                                        in_=py,
                                    )
                                else:
                                    nc.vector.tensor_copy(
                                        y2[:, ech * 512 : (ech + 1) * 512], py
                                    )
                            else:
                                nc.vector.tensor_copy(
                                    y2[:, ech * 512 : (ech + 1) * 512], py
                                )
                        nc.sync.dma_start(
                            out=out2_d[ct, (i - 8) * 128 : (i - 7) * 128, :],
                            in_=y2,
                        )


def build_nc():
    _install_drain_fix()
    from contextlib import ExitStack

    nc = bacc.Bacc()
    with ExitStack() as ctx:
        tc = ctx.enter_context(tile.TileContext(nc))
        _emit(nc, tc, ctx)
    nc.finalize()  # Bacc: alloc_regs + insert_library_loads happen here
    return nc


def make_in_maps(inputs, w_attn, b_attn, w_proj, b_proj):
    """Build the 8 per-core input dicts from the full tensors.
    X / weights / mask go down pre-converted to bf16 (the compute dtype)."""
    x = np.asarray(inputs, dtype=np.float32)
    w_attn = np.asarray(w_attn, dtype=np.float32)
    b_attn = np.asarray(b_attn, dtype=np.float32)
    w_proj = np.asarray(w_proj, dtype=np.float32)
    b_proj = np.asarray(b_proj, dtype=np.float32)

    in_maps = []
    for c in range(8):
        b, half = c // 2, c % 2
        h0 = half * 8
        cols = np.arange(h0 * 64, h0 * 64 + 512)
        wqk = np.ascontiguousarray(
            np.concatenate([w_attn[:, cols], w_attn[:, 1024 + cols]], axis=1).astype(
                BF16_NP
            )
        )
        bqk = np.concatenate([b_attn[cols], b_attn[1024 + cols]]).reshape(8, 128, 1)
        vbase = 2048 + h0 * 64
        wva = w_attn[:, vbase : vbase + 512]
        bva = b_attn[vbase : vbase + 512].reshape(1, 512)
        wp = np.ascontiguousarray(w_proj[h0 * 64 : h0 * 64 + 512, :].astype(BF16_NP))
        bp = (b_proj if half == 0 else np.zeros_like(b_proj)).reshape(1, E)
        in_maps.append(
            {
                "xt": np.ascontiguousarray(x[b].T.astype(BF16_NP)),
                "wqk": wqk,
                "wva": np.ascontiguousarray(wva.astype(BF16_NP)),
                "wp": wp,
                "bqk": np.ascontiguousarray(bqk.astype(np.float32)),
                "bva": bva,
                "bp": np.ascontiguousarray(bp.astype(np.float32)),
            }
        )
    return in_maps


_CACHE = {}


def kernel(**inputs):
    nc = _CACHE.get("nc")
    if nc is None:
        nc = _CACHE["nc"] = build_nc()
    in_maps = make_in_maps(
        inputs["inputs"],
        inputs["w_attn"],
        inputs["b_attn"],
        inputs["w_proj"],
        inputs["b_proj"],
    )
    res = run_bass_kernel_spmd(nc, in_maps, core_ids=list(range(8)))
    return gather(res.results)


def gather(results):
    out = np.zeros((4, S, E), dtype=np.float32)
    for b in range(4):
        for c in (2 * b, 2 * b + 1):
            r = results[c]
            # rows 0:1024 come from "out"; the device writes rows 1024:2048
            # only via the per-head-pair partials in "out2"
            out[b, 0:1024] += r["out"][0:1024].astype(np.float32)
            out[b, 1024:2048] += r["out2"].astype(np.float32).sum(axis=0)
    return out

